# revision 1
# baseline (speedup 1.0000x reference)
"""v2 KNN kernel: 3-pass float32r matmul (exact fp32 via 12-bit hi/lo split).

Same algorithm as kernel.py v1, but each fp32 operand is pre-split into
hi (mantissa bits 23..12, fp22-exact) + lo (bits 11..0, also fp22-exact);
cross = hi.hi + hi.lo + lo.hi (lo.lo ~ 2^-24, dropped) runs as three
1-cycle/row float32r passes instead of one 4-cycle/row fp32 pass.
SBUF doubling for hi/lo is paid for by processing the candidate shard in
two 4096-wide halves with rhs re-DMA'd per half.
"""

import numpy as np
import ml_dtypes

N_CORES = 8
NQ = 4096
NTRAIN = 65536
KDIM = 256
SHARD = NTRAIN // N_CORES   # 8192
HALF = SHARD // 2           # 4096
P = 128
NQT = NQ // P               # 32
CT = 512
NCT_H = HALF // CT          # 8 per half
CHUNK = 1024
NCHUNK_H = HALF // CHUNK    # 4 per half
KC = KDIM // P              # 2
TOPK = 16

_CACHE = {}


def _build_program(nqt=NQT):
    import concourse.mybir as mybir
    import concourse.tile as tile
    from concourse import bacc

    NQT = nqt  # noqa: N806
    NQ = NQT * P  # noqa: N806

    nc = bacc.Bacc(
        "TRN2", target_bir_lowering=False, debug=False, enable_asserts=False
    )
    f32 = mybir.dt.float32
    f32r = mybir.dt.float32r
    bf16 = mybir.dt.bfloat16
    u32 = mybir.dt.uint32

    lh = nc.dram_tensor("lh", [P, KC, NQ], f32r, kind="ExternalInput").ap()
    ll = nc.dram_tensor("ll", [P, KC, NQ], f32r, kind="ExternalInput").ap()
    rh = nc.dram_tensor("rh", [P, KC, SHARD], f32r, kind="ExternalInput").ap()
    rl = nc.dram_tensor("rl", [P, KC, SHARD], f32r, kind="ExternalInput").ap()
    tsp = nc.dram_tensor("tsp", [P, SHARD], bf16, kind="ExternalInput").ap()
    ones = nc.dram_tensor("ones", [P, P], bf16, kind="ExternalInput").ap()
    ovals = nc.dram_tensor("ovals", [2, NQT, P, 32], f32, kind="ExternalOutput").ap()
    oidx = nc.dram_tensor("oidx", [2, NQT, P, 32], u32, kind="ExternalOutput").ap()

    with tile.TileContext(nc) as tc:
        with (
            tc.tile_pool(name="const", bufs=1) as cpool,
            tc.tile_pool(name="rhs", bufs=1) as rpool,
            tc.tile_pool(name="scores", bufs=2) as spool,
            tc.tile_pool(name="outs", bufs=2) as opool,
            tc.tile_pool(name="psum", bufs=8, space="PSUM") as ppool,
        ):
            lh_sb = cpool.tile([P, KC, NQ], f32r)
            ll_sb = cpool.tile([P, KC, NQ], f32r)
            tsp_sb = cpool.tile([P, SHARD], bf16)
            ones_sb = cpool.tile([P, P], bf16)
            nc.sync.dma_start(ones_sb[:], ones[:])
            nc.sync.dma_start(tsp_sb[:], tsp[:])
            for kc in range(KC):
                nc.sync.dma_start(lh_sb[:, kc], lh[:, kc])
                nc.sync.dma_start(ll_sb[:, kc], ll[:, kc])

            for half in range(2):
                hsl = slice(half * HALF, (half + 1) * HALF)
                rh_sb = rpool.tile([P, KC, HALF], f32r, tag="rh")
                rl_sb = rpool.tile([P, KC, HALF], f32r, tag="rl")
                for kc in range(KC):
                    nc.sync.dma_start(rh_sb[:, kc], rh[:, kc, hsl])
                    nc.sync.dma_start(rl_sb[:, kc], rl[:, kc, hsl])

                for qt in range(NQT):
                    scores = spool.tile([P, HALF], f32, tag="scores")
                    for ct in range(NCT_H):
                        pt = ppool.tile([P, CT], f32, tag="ps")
                        csl = slice(ct * CT, (ct + 1) * CT)
                        gsl = slice(half * HALF + ct * CT,
                                    half * HALF + (ct + 1) * CT)
                        nc.tensor.matmul(
                            pt[:], ones_sb[:], tsp_sb[:, gsl],
                            start=True, stop=False,
                        )
                        qsl = slice(qt * P, (qt + 1) * P)
                        for kc in range(KC):
                            last = kc == KC - 1
                            nc.tensor.matmul(
                                pt[:], lh_sb[:, kc, qsl], rh_sb[:, kc, csl],
                                start=False, stop=False,
                            )
                            nc.tensor.matmul(
                                pt[:], lh_sb[:, kc, qsl], rl_sb[:, kc, csl],
                                start=False, stop=False,
                            )
                            nc.tensor.matmul(
                                pt[:], ll_sb[:, kc, qsl], rh_sb[:, kc, csl],
                                start=False, stop=last,
                            )
                        nc.scalar.copy(scores[:, csl], pt[:])
                    vals = opool.tile([P, 32], f32, tag="vals")
                    idxs = opool.tile([P, 32], u32, tag="idxs")
                    for ch in range(NCHUNK_H):
                        ssl = scores[:, ch * CHUNK : (ch + 1) * CHUNK]
                        osl = slice(ch * 8, (ch + 1) * 8)
                        nc.vector.max(out=vals[:, osl], in_=ssl)
                        nc.vector.max_index(
                            out=idxs[:, osl], in_max=vals[:, osl], in_values=ssl
                        )
                    nc.sync.dma_start(ovals[half, qt], vals[:])
                    nc.sync.dma_start(oidx[half, qt], idxs[:])

    nc.compile()
    return nc


def _split_bf16x3(v):
    a = v.astype(ml_dtypes.bfloat16)
    r1 = (v - a.astype(np.float32)).astype(np.float32)
    b = r1.astype(ml_dtypes.bfloat16)
    r2 = (r1 - b.astype(np.float32)).astype(np.float32)
    c = r2.astype(ml_dtypes.bfloat16)
    return a, b, c


def _hilo(x):
    hi = (x.view(np.uint32) & np.uint32(0xFFFFF000)).view(np.float32)
    lo = (x - hi).astype(np.float32)
    return np.ascontiguousarray(hi), np.ascontiguousarray(lo)


def _prep_inputs(X_train, X_test):
    X_train = np.asarray(X_train, dtype=np.float32)
    X_test = np.asarray(X_test, dtype=np.float32)

    lhsT = np.ascontiguousarray(X_test.T.reshape(KC, P, NQ).transpose(1, 0, 2))
    lh, ll = _hilo(lhsT)
    ones_np = np.zeros((P, P), dtype=ml_dtypes.bfloat16)
    ones_np[0:3] = 1.0

    in_maps = []
    for core in range(N_CORES):
        shard = X_train[core * SHARD : (core + 1) * SHARD]
        rhsT = np.ascontiguousarray(
            (2.0 * shard.T).reshape(KC, P, SHARD).transpose(1, 0, 2)
        )
        rh, rl = _hilo(rhsT)
        neg_sq = -np.sum(shard * shard, axis=1, dtype=np.float32)
        a, b, c = _split_bf16x3(neg_sq)
        tsp_np = np.zeros((P, SHARD), dtype=ml_dtypes.bfloat16)
        tsp_np[0], tsp_np[1], tsp_np[2] = a, b, c
        in_maps.append(
            {"lh": lh, "ll": ll, "rh": rh, "rl": rl, "tsp": tsp_np,
             "ones": ones_np}
        )
    return in_maps


def _merge_topk(results):
    ncand = 2 * NCHUNK_H * 8  # 64 per core
    all_vals = np.empty((NQ, N_CORES * ncand), dtype=np.float32)
    all_gidx = np.empty((NQ, N_CORES * ncand), dtype=np.int64)
    # candidate j of (half, chunk, rank): base = half*HALF + chunk*CHUNK
    base = (
        np.repeat(np.arange(2, dtype=np.int64) * HALF, NCHUNK_H * 8)
        + np.tile((np.arange(NCHUNK_H * 8, dtype=np.int64) // 8) * CHUNK, 2)
    )
    for core in range(N_CORES):
        vals = results[core]["ovals"].transpose(1, 2, 0, 3).reshape(NQ, ncand)
        idxs = (
            results[core]["oidx"].transpose(1, 2, 0, 3).reshape(NQ, ncand)
            .astype(np.int64)
        )
        # after transpose, candidate axis is (half, 32) flattened per query
        gidx = idxs + base[None, :] + core * SHARD
        sl = slice(core * ncand, (core + 1) * ncand)
        all_vals[:, sl] = vals
        all_gidx[:, sl] = gidx

    order_idx = np.argsort(all_gidx, axis=1, kind="stable")
    v = np.take_along_axis(all_vals, order_idx, axis=1)
    g = np.take_along_axis(all_gidx, order_idx, axis=1)
    order_val = np.argsort(-v, axis=1, kind="stable")[:, :TOPK]
    return np.take_along_axis(g, order_val, axis=1).astype(np.int32)


def _get_nc():
    if "nc" not in _CACHE:
        _CACHE["nc"] = _build_program()
    return _CACHE["nc"]


def kernel(X_train, X_test):
    from concourse.bass_utils import run_bass_kernel_spmd

    nc = _get_nc()
    in_maps = _prep_inputs(X_train, X_test)
    last_err = None
    for _attempt in range(3):
        try:
            res = run_bass_kernel_spmd(
                nc, in_maps, core_ids=list(range(N_CORES))
            )
            break
        except Exception as e:
            last_err = e
    else:
        raise last_err
    return _merge_topk(res.results)

